# revision 13
# baseline (speedup 1.0000x reference)
"""Trainium2 Bass kernel for nn_MultiHeadAttention (LN -> QKV -> MHA -> FC -> +residual).

Sharding: data-parallel over batch (B=8 -> 1 batch element per NeuronCore).
Returns (out, attn) matching the jax reference.

Per-core pipeline (T=1024, C=1024, H=16, Dk=64):
  A. LN stats (bn_stats) + normalize in [T,C]; PE-transpose tiles -> h^T [C,T]
     with ln_g/ln_b affine fused into the PSUM evacuation (output f32r).
  B. q^T,k^T = (W_qkv as stationary).T @ h^T  -> [2048, T] f32r
     v       = h^T as stationary @ W_v        -> [T, 1024] natural, stored bf16
     with a ones column appended per head (flash denominator trick).
  C. per head: S  = q^T.T @ k^T   (natural)  -> exp via ACT with accum_out
                  -> row sums l; attn_out = P * (1/l)  (DMA to HBM)
               S^T = k^T.T @ q^T  (transposed) -> exp -> P^T bf16
               O'^T = v_aug.T @ P^T  (65 rows: 64 of O^T + row of l)
                  -> normalize O^T columns via PE-broadcast 1/l row
  D. out = O^T.T @ W_fc + b_fc + x   (residual+bias fused into evacuation)
"""
import sys

sys.path.insert(0, '/opt/trn_rl_repo')

import numpy as np

import concourse.bacc as bacc
import concourse.bass as bass
import concourse.mybir as mybir
import concourse.tile as tile
from concourse.bass_utils import run_bass_kernel_spmd
from concourse.masks import make_identity

F32 = mybir.dt.float32
F32R = mybir.dt.float32r
BF16 = mybir.dt.bfloat16
AF = mybir.ActivationFunctionType
OP = mybir.AluOpType

B, T, C = 8, 1024, 1024
H, DK = 16, 64
KO = C // 128          # 8 contraction chunks
TT = T // 128          # 8 token tiles
LN_EPS = 1e-5
SCALE = 1.0 / 8.0      # 1/sqrt(DK)
N_CORES = 8


def build_nc(trace_label=""):
    nc = bacc.Bacc("TRN2", target_bir_lowering=False, debug=False,
                   num_devices=N_CORES)
    x = nc.dram_tensor("x", [T, C], F32, kind="ExternalInput")
    ln_g = nc.dram_tensor("ln_g", [C], F32, kind="ExternalInput")
    ln_b = nc.dram_tensor("ln_b", [C], F32, kind="ExternalInput")
    w_qkv = nc.dram_tensor("w_qkv", [C, 3 * C], F32, kind="ExternalInput")
    b_qkv = nc.dram_tensor("b_qkv", [3 * C], F32, kind="ExternalInput")
    w_fc = nc.dram_tensor("w_fc", [C, C], F32, kind="ExternalInput")
    b_fc = nc.dram_tensor("b_fc", [C], F32, kind="ExternalInput")
    out = nc.dram_tensor("out", [T, C], F32, kind="ExternalOutput")
    attn = nc.dram_tensor("attn", [H, T, T], F32, kind="ExternalOutput")

    with tile.TileContext(nc) as tc:
        build_body(nc, tc, x, ln_g, ln_b, w_qkv, b_qkv, w_fc, b_fc, out, attn)
    nc.compile()
    return nc


def build_body(nc, tc, x, ln_g, ln_b, w_qkv, b_qkv, w_fc, b_fc, out, attn):
    from contextlib import ExitStack

    P = 128

    singles_cm = tc.tile_pool(name="singles", bufs=1)
    singles = singles_cm.__enter__()

    ident = singles.tile([P, P], F32)
    make_identity(nc, ident)

    eps_sb = singles.tile([P, 1], F32)
    nc.vector.memset(eps_sb, LN_EPS)

    # per-chunk ln scale/bias columns: g_sb[:, j] = ln_g[j*128:(j+1)*128]
    g_sb = singles.tile([P, KO], F32)
    b_sb = singles.tile([P, KO], F32)
    nc.sync.dma_start(g_sb, ln_g.rearrange("(o p) -> p o", p=P))
    nc.sync.dma_start(b_sb, ln_b.rearrange("(o p) -> p o", p=P))

    # qk bias columns (features 0..2047)
    bqk_sb = singles.tile([P, 16], F32)
    nc.sync.dma_start(bqk_sb, b_qkv[0:2 * C].rearrange("(o p) -> p o", p=P))

    # resident big tensors
    hT = singles.tile([P, KO, T], F32R)         # h^T   (freed logically after B)
    qkT = singles.tile([P, 16, T], F32R)        # q^T,k^T stacked along m
    v_sb = singles.tile([P, TT, H, DK + 1], BF16)  # v natural + ones column
    oT = singles.tile([P, KO, T], F32R)         # O^T (normalized)

    nc.gpsimd.memset(v_sb[:, :, :, DK:DK + 1], 1.0)

    # ones row at partition 64 (for the 1/l broadcast matmul)
    ones_sb = singles.tile([65, DK], BF16)
    nc.gpsimd.memset(ones_sb[64:65, :], 1.0)

    # ---------------- Stage A: LN + transpose ----------------
    with ExitStack() as stA:
        pa = stA.enter_context(tc.tile_pool(name="pa", bufs=3))
        pax = stA.enter_context(tc.tile_pool(name="pax", bufs=1))
        psA = stA.enter_context(tc.tile_pool(name="psA", bufs=4, space="PSUM"))
        x_sb = pax.tile([P, TT, C], F32)
        nc.sync.dma_start(x_sb, x.rearrange("(ti p) c -> p ti c", p=P))
        for i in range(TT):
            stats = pa.tile([P, 2, 6], F32, tag="stats")
            nc.vector.bn_stats(out=stats[:, 0, :], in_=x_sb[:, i, 0:512])
            nc.vector.bn_stats(out=stats[:, 1, :], in_=x_sb[:, i, 512:1024])
            mv = pa.tile([P, 2], F32, tag="mv")
            nc.vector.bn_aggr(out=mv, in_=stats)
            std = pa.tile([P, 1], F32, tag="std")
            nc.scalar.activation(std, mv[:, 1:2], AF.Sqrt, bias=eps_sb)
            rstd = pa.tile([P, 1], F32, tag="rstd")
            nc.vector.reciprocal(rstd, std)
            xhat = pa.tile([P, C], F32, tag="xhat")
            nc.vector.tensor_scalar(xhat, x_sb[:, i], mv[:, 0:1], rstd,
                                    OP.subtract, OP.mult)
            for j in range(KO):
                ps_t = psA.tile([P, P], F32, tag="tr")
                nc.tensor.transpose(ps_t, xhat[:, j * P:(j + 1) * P], ident)
                # hT[j][:, i-range] = ps_t * g[j] + b[j]  (rounds to f32r)
                nc.vector.tensor_scalar(hT[:, j, i * P:(i + 1) * P], ps_t,
                                        g_sb[:, j:j + 1], b_sb[:, j:j + 1],
                                        OP.mult, OP.add)

    # ---------------- Stage B: QKV ----------------
    with ExitStack() as stB:
        pw = stB.enter_context(tc.tile_pool(name="pw", bufs=2))
        pwv = stB.enter_context(tc.tile_pool(name="pwv", bufs=1))
        psB = stB.enter_context(tc.tile_pool(name="psB", bufs=4, space="PSUM"))

        # v-bias broadcast to all 128 partitions via DMA replicate
        bv_bcast = pwv.tile([P, C], F32)
        bv_ap = bass.AP(tensor=b_qkv.ap().tensor, offset=2 * C, ap=[[0, P], [1, C]])
        nc.gpsimd.dma_start(out=bv_bcast, in_=bv_ap)

        # q^T, k^T : feature m-tiles as output partitions
        for m in range(16):
            wq_raw = pw.tile([P, KO, P], F32, tag="wqraw")
            nc.sync.dma_start(
                wq_raw,
                w_qkv.rearrange("(ko ki) f -> ki ko f", ki=P)[:, :, m * P:(m + 1) * P])
            wq_r = pw.tile([P, KO, P], F32R, tag="wqr")
            nc.gpsimd.tensor_copy(wq_r, wq_raw)
            for half in range(2):
                ps_qk = psB.tile([P, 512], F32, tag="qk")
                for j in range(KO):
                    nc.tensor.matmul(ps_qk, wq_r[:, j],
                                     hT[:, j, half * 512:(half + 1) * 512],
                                     start=(j == 0), stop=(j == KO - 1))
                nc.vector.tensor_scalar(qkT[:, m, half * 512:(half + 1) * 512],
                                        ps_qk, bqk_sb[:, m:m + 1], None, OP.add)

        # v natural: [t, feat] tiles
        wv_r = pwv.tile([P, KO, C], F32R)
        for j in range(KO):
            wv_raw = pw.tile([P, C], F32, tag="wvraw")
            nc.sync.dma_start(wv_raw, w_qkv[j * P:(j + 1) * P, 2 * C:3 * C])
            nc.gpsimd.tensor_copy(wv_r[:, j], wv_raw)
        for i in range(TT):
            for half in range(2):
                ps_v = psB.tile([P, 512], F32, tag="v")
                for j in range(KO):
                    nc.tensor.matmul(ps_v, hT[:, j, i * P:(i + 1) * P],
                                     wv_r[:, j, half * 512:(half + 1) * 512],
                                     start=(j == 0), stop=(j == KO - 1))
                # bias-add + scatter into v_sb[:, i, h0:h0+8, 0:64] (bf16) in one op
                h0 = half * 8
                nc.vector.tensor_tensor(
                    v_sb[:, i, h0:h0 + 8, 0:DK],
                    ps_v.rearrange("p (h d) -> p h d", h=8),
                    bv_bcast[:, half * 512:(half + 1) * 512].rearrange(
                        "p (h d) -> p h d", h=8),
                    OP.add)

    # ---------------- Stage C: attention per head ----------------
    with ExitStack() as stC:
        pc = stC.enter_context(tc.tile_pool(name="pc", bufs=2))
        ppt = stC.enter_context(tc.tile_pool(name="ppt", bufs=2))
        psS = stC.enter_context(tc.tile_pool(name="psS", bufs=2, space="PSUM"))
        psO = stC.enter_context(tc.tile_pool(name="psO", bufs=2, space="PSUM"))

        for h in range(H):
            qm, qp = h // 2, 64 * (h % 2)
            km, kp = 8 + h // 2, 64 * (h % 2)

            # --- natural S -> attn output ---
            for i in range(TT):
                ps_s = psS.tile([P, T], F32, tag="s")
                for half in range(2):
                    nc.tensor.matmul(
                        ps_s[:, half * 512:(half + 1) * 512],
                        qkT[qp:qp + 64, qm, i * P:(i + 1) * P],
                        qkT[kp:kp + 64, km, half * 512:(half + 1) * 512],
                        start=True, stop=True)
                pnat = pc.tile([P, T], F32, tag="pnat")
                lcol = pc.tile([P, 1], F32, tag="lcol")
                nc.scalar.activation(pnat, ps_s, AF.Exp, scale=SCALE,
                                     accum_out=lcol)
                rcol = pc.tile([P, 1], F32, tag="rcol")
                nc.vector.reciprocal(rcol, lcol)
                asb = pc.tile([P, T], F32, tag="attn")
                nc.vector.tensor_scalar(asb, pnat, rcol, None, OP.mult)
                nc.scalar.dma_start(attn[h, i * P:(i + 1) * P, :], asb)

            # --- transposed S -> P^T (bf16) ---
            PT = ppt.tile([P, TT, T], BF16, tag="pt")
            for i in range(TT):
                ps_st = psS.tile([P, T], F32, tag="s")
                for half in range(2):
                    nc.tensor.matmul(
                        ps_st[:, half * 512:(half + 1) * 512],
                        qkT[kp:kp + 64, km, i * P:(i + 1) * P],
                        qkT[qp:qp + 64, qm, half * 512:(half + 1) * 512],
                        start=True, stop=True)
                nc.scalar.activation(PT[:, i], ps_st, AF.Exp, scale=SCALE)

            # --- P^T @ v -> O'^T (65 rows) + normalize ---
            for n in range(2):
                ps_o = psO.tile([P, 512], F32, tag="o")
                for i in range(TT):
                    nc.tensor.matmul(ps_o[0:65],
                                     v_sb[:, i, h, :],
                                     PT[:, i, n * 512:(n + 1) * 512],
                                     start=(i == 0), stop=(i == TT - 1))
                rrow_f = pc.tile([65, 512], F32, tag="rrowf")
                nc.vector.reciprocal(rrow_f[64:65, :], ps_o[64:65, :])
                rrow = pc.tile([65, 512], BF16, tag="rrow")
                nc.vector.tensor_copy(rrow[64:65, :], rrow_f[64:65, :])
                ps_b = psO.tile([64, 512], F32, tag="bc")
                nc.tensor.matmul(ps_b, ones_sb[64:65, :], rrow[64:65, :],
                                 start=True, stop=True)
                bc_sb = pc.tile([64, 512], F32, tag="bcsb")
                nc.vector.tensor_copy(bc_sb, ps_b)
                nc.vector.tensor_tensor(
                    oT[qp:qp + 64, h // 2, n * 512:(n + 1) * 512],
                    ps_o[0:64], bc_sb, OP.mult)

    # ---------------- Stage D: FC + residual ----------------
    with ExitStack() as stD:
        pd = stD.enter_context(tc.tile_pool(name="pd", bufs=2))
        pwf = stD.enter_context(tc.tile_pool(name="pwf", bufs=1))
        psD = stD.enter_context(tc.tile_pool(name="psD", bufs=4, space="PSUM"))

        bfc_bcast = pwf.tile([P, C], F32)
        bfc_ap = bass.AP(tensor=b_fc.ap().tensor, offset=0, ap=[[0, P], [1, C]])
        nc.gpsimd.dma_start(out=bfc_bcast, in_=bfc_ap)

        wf_r = pwf.tile([P, KO, C], F32R)
        for j in range(KO):
            wf_raw = pd.tile([P, C], F32, tag="wfraw")
            nc.sync.dma_start(wf_raw, w_fc[j * P:(j + 1) * P, :])
            nc.gpsimd.tensor_copy(wf_r[:, j], wf_raw)

        for i in range(TT):
            x_re = pd.tile([P, C], F32, tag="xre")
            nc.sync.dma_start(x_re, x[i * P:(i + 1) * P, :])
            # fold b_fc into the residual in place
            nc.vector.tensor_tensor(x_re, x_re, bfc_bcast, OP.add)
            osb = pd.tile([P, C], F32, tag="osb")
            for half in range(2):
                ps_fc = psD.tile([P, 512], F32, tag="fc")
                for j in range(KO):
                    nc.tensor.matmul(ps_fc, oT[:, j, i * P:(i + 1) * P],
                                     wf_r[:, j, half * 512:(half + 1) * 512],
                                     start=(j == 0), stop=(j == KO - 1))
                nc.vector.tensor_tensor(osb[:, half * 512:(half + 1) * 512],
                                        ps_fc,
                                        x_re[:, half * 512:(half + 1) * 512],
                                        OP.add)
            nc.sync.dma_start(out[i * P:(i + 1) * P, :], osb)

    singles_cm.__exit__(None, None, None)


_NC_CACHE = None


def kernel(x, ln_g, ln_b, W_qkv, b_qkv, W_fc, b_fc):
    global _NC_CACHE
    if _NC_CACHE is None:
        _NC_CACHE = build_nc()
    nc = _NC_CACHE

    x = np.ascontiguousarray(np.asarray(x, dtype=np.float32))
    shared = {
        "ln_g": np.ascontiguousarray(np.asarray(ln_g, np.float32)),
        "ln_b": np.ascontiguousarray(np.asarray(ln_b, np.float32)),
        "w_qkv": np.ascontiguousarray(np.asarray(W_qkv, np.float32)),
        "b_qkv": np.ascontiguousarray(np.asarray(b_qkv, np.float32)),
        "w_fc": np.ascontiguousarray(np.asarray(W_fc, np.float32)),
        "b_fc": np.ascontiguousarray(np.asarray(b_fc, np.float32)),
    }
    in_maps = [{"x": x[b], **shared} for b in range(B)]
    res = run_bass_kernel_spmd(nc, in_maps, core_ids=list(range(N_CORES)))
    out = np.stack([res.results[b]["out"] for b in range(B)])
    attn = np.stack([res.results[b]["attn"] for b in range(B)])
    return out, attn
